# revision 1
# baseline (speedup 1.0000x reference)
"""Conv2D 3x3 (stride 1, pad 1) Trainium2 Bass kernel.

Problem: x (16,128,56,56) f32  *  W (256,128,3,3) f32  + b (256,)  ->  (16,256,56,56) f32

Strategy:
  - Data parallel over batch: 8 NeuronCores x 2 images each; W/b replicated.
  - Host pre-pads each image with a 1-pixel zero halo (58x58) so every kernel
    tap is a pure strided SBUF read -- no edge cases on device.
  - Implicit GEMM: contraction over C_IN=128 (the SBUF partition dim); for each
    output tile of 8 rows (N = 8*56 = 448 pixels, one PSUM bank) accumulate the
    9 taps as 9 matmuls: psum[co,pix] += W[ci,co,tap].T @ xpad[ci,shifted pix].
  - bf16 inputs (tolerance 2e-2 >> bf16 conv err ~2.4e-3): enables the PE's
    fast weight load (FWL, ~27ns vs ~195ns for fp32) and halves input DMA.
  - Chunk-major weight layout + split x DMAs so the first matmul only waits on
    ~0.5 MB of DMA; dummy warmup matmuls keep the PE HAM window busy meanwhile.
  - PSUM -> SBUF eviction + per-channel bias via one ScalarE activation.
"""

import os
import sys

for _p in ("/opt/trn_rl_repo", os.path.expanduser("~/.axon_site/_ro/trn_rl_repo")):
    if os.path.isdir(_p) and _p not in sys.path:
        sys.path.insert(0, _p)
        break

import numpy as np
import ml_dtypes

B, C_IN, H, W_SP = 16, 128, 56, 56
C_OUT, KH, KW = 256, 3, 3
N_CORES = 8
B_PER_CORE = B // N_CORES          # 2
CHUNKS = C_OUT // 128              # 2 chunks of 128 output channels
HP, WP = H + 2, W_SP + 2           # 58x58 padded image
ROWS_PER_TILE = 8                  # 8*56 = 448 <= 512 fp32 / PSUM bank
N_TILE = ROWS_PER_TILE * W_SP      # 448
H_TILES = H // ROWS_PER_TILE       # 7
W_PER_CHUNK = KH * KW * 128        # 1152 weight columns per chunk
N_WARM = 34                        # small-N dummy matmuls to pre-warm the PE clock

# x DMA piece boundaries (padded rows). Tile ht needs rows ht*8 .. ht*8+9.
# Image 0 pieces are sized to per-tile needs so the first groups unblock early.
X_ROW_SPLITS0 = (0, 10, 18, 26, 42, HP)
X_ROW_SPLITS = (0, 16, 30, 44, HP)

_CACHE = {}


def _build(repeat=1, tag=0, null=False):
    from concourse import bacc, mybir
    import concourse.tile as tile

    f32 = mybir.dt.float32
    bf16 = mybir.dt.bfloat16

    nc = bacc.Bacc(trn_type="TRN2", name="conv2d_dp")
    x_h = nc.dram_tensor("x", [B_PER_CORE, C_IN, HP * WP], bf16, kind="ExternalInput")
    # wt layout: [ci, chunk*1152 + (kh*3+kw)*128 + co_mod]  (chunk-major so the
    # first chunk's weights land before the second's)
    # `tag` pads the wt free dim so benchmark variants hash differently in the
    # (BIR-payload-blind) neuron compile cache.
    w_h = nc.dram_tensor("wt", [C_IN, CHUNKS * W_PER_CHUNK + tag], bf16,
                         kind="ExternalInput")
    # bias layout: [co_mod, chunk]
    b_h = nc.dram_tensor("bias", [128, CHUNKS], f32, kind="ExternalInput")
    o_h = nc.dram_tensor("out", [B_PER_CORE, C_OUT, H, W_SP], f32, kind="ExternalOutput")

    with tile.TileContext(nc) as tc:
        with tc.tile_pool(name="const", bufs=1) as cpool, \
             tc.tile_pool(name="xs", bufs=2) as xpool, \
             tc.tile_pool(name="os", bufs=4) as opool, \
             tc.tile_pool(name="ps", bufs=8, space="PSUM") as ppool:
            b_sb = cpool.tile([128, CHUNKS], f32)

            if null:
                # timing-overhead probe: same I/O signature, near-zero work
                nc.sync.dma_start(out=b_sb[:, :], in_=b_h[:, :])
                zt = cpool.tile([128, N_TILE], f32)
                nc.vector.memset(zt[:, :], 0)
                nc.sync.dma_start(out=o_h[0, :128, :ROWS_PER_TILE, :], in_=zt[:, :])
                nc.finalize()
                return nc

            # PE warmup: small-N dummy matmuls on a zeroed tile into a scratch
            # PSUM bank, runnable right after the start barrier (no DMA
            # dependency). They overlap the initial weight/x DMA and open the
            # HAM activity window early; N=128 keeps the bridge granularity
            # fine so real matmuls start within ~100ns of their data landing.
            warm = cpool.tile([128, N_TILE], bf16)
            nc.gpsimd.memset(warm[:, :128], 0)
            wps = ppool.tile([128, N_TILE], f32, name="ps")
            for _ in range(N_WARM):
                nc.tensor.matmul(wps[:, :128], warm[:, :128], warm[:, :128],
                                 start=True, stop=True)

            w_sb = cpool.tile([C_IN, CHUNKS * W_PER_CHUNK], bf16)
            o_qs = (nc.sync, nc.scalar, nc.sync)

            for rep in range(repeat):
              xps = [xpool.tile([C_IN, HP * WP], bf16, name=f"xp{b}")
                     for b in range(B_PER_CORE)]

              # The first weight taps and first x piece gate the first real
              # matmul. Per-queue DMA throughput is the startup bottleneck
              # during the 8-core burst, so spread the critical transfers
              # across all three DMA-capable queues, each leading its queue:
              #   Sync (HWDGE):  w chunk0 taps 0-5
              #   Scalar:        w chunk0 taps 6-8, bias, w chunk1
              #   GpSimd (SWDGE): x pieces in need order
              nc.sync.dma_start(out=w_sb[:, :6 * 128], in_=w_h[:, :6 * 128])
              nc.scalar.dma_start(out=w_sb[:, 6 * 128:W_PER_CHUNK],
                                  in_=w_h[:, 6 * 128:W_PER_CHUNK])
              nc.scalar.dma_start(out=b_sb[:, :], in_=b_h[:, :])
              nc.scalar.dma_start(out=w_sb[:, W_PER_CHUNK:],
                                  in_=w_h[:, W_PER_CHUNK:2 * W_PER_CHUNK])
              for b in range(B_PER_CORE):
                  splits = X_ROW_SPLITS0 if b == 0 else X_ROW_SPLITS
                  for r0, r1 in zip(splits, splits[1:]):
                      nc.gpsimd.dma_start(out=xps[b][:, r0 * WP:r1 * WP],
                                          in_=x_h[b, :, r0 * WP:r1 * WP])

              for b in range(B_PER_CORE):
                xp = xps[b]
                x3 = xp.rearrange("p (r c) -> p r c", r=HP)

                for chunk in range(CHUNKS):
                    for ht in range(H_TILES):
                        h0 = ht * ROWS_PER_TILE
                        ps = ppool.tile([128, N_TILE], f32, name="ps")
                        for tap in range(KH * KW):
                            dh, dw = divmod(tap, KW)
                            wcol = chunk * W_PER_CHUNK + tap * 128
                            nc.tensor.matmul(
                                ps[:, :],
                                w_sb[:, wcol:wcol + 128],
                                x3[:, h0 + dh:h0 + dh + ROWS_PER_TILE, dw:dw + W_SP],
                                start=(tap == 0),
                                stop=(tap == KH * KW - 1),
                            )
                        osb = opool.tile([128, N_TILE], f32, name="osb")
                        last = (b == B_PER_CORE - 1 and chunk == CHUNKS - 1
                                and ht == H_TILES - 1)
                        if last:
                            # split the final eviction so its PSUM->SBUF copy
                            # and DMA pipeline (shortens the kernel tail)
                            half = N_TILE // 2
                            for hi, q in ((0, nc.sync), (1, nc.scalar)):
                                nc.scalar.activation(
                                    osb[:, hi * half:(hi + 1) * half],
                                    ps[:, hi * half:(hi + 1) * half],
                                    mybir.ActivationFunctionType.Identity,
                                    bias=b_sb[:, chunk:chunk + 1],
                                )
                                q.dma_start(
                                    out=o_h[b, chunk * 128:(chunk + 1) * 128,
                                            h0 + hi * ROWS_PER_TILE // 2:
                                            h0 + (hi + 1) * ROWS_PER_TILE // 2, :],
                                    in_=osb[:, hi * half:(hi + 1) * half],
                                )
                        else:
                            nc.scalar.activation(
                                osb[:, :], ps[:, :],
                                mybir.ActivationFunctionType.Identity,
                                bias=b_sb[:, chunk:chunk + 1],
                            )
                            o_qs[(b * CHUNKS * H_TILES + chunk * H_TILES + ht) % 3].dma_start(
                                out=o_h[b, chunk * 128:(chunk + 1) * 128,
                                        h0:h0 + ROWS_PER_TILE, :],
                                in_=osb[:, :],
                            )
    nc.finalize()
    return nc


def _get_nc(repeat=1, tag=0, null=False):
    key = ("nc", repeat, tag, null)
    if key not in _CACHE:
        _CACHE[key] = _build(repeat, tag=tag, null=null)
    return _CACHE[key]


def kernel(x, W, b, _trace=False):
    from concourse.bass_utils import run_bass_kernel_spmd

    x = np.asarray(x, dtype=np.float32)
    W = np.asarray(W, dtype=np.float32)
    b = np.asarray(b, dtype=np.float32)

    bf16 = ml_dtypes.bfloat16
    # zero-pad spatial dims to 58x58 on host, flatten, cast to bf16
    xpad = np.zeros((B, C_IN, HP, WP), dtype=bf16)
    xpad[:, :, 1:1 + H, 1:1 + W_SP] = x.astype(bf16)
    xpad = np.ascontiguousarray(xpad.reshape(B, C_IN, HP * WP))

    # [co,ci,kh,kw] -> [ci, chunk*1152 + (kh*3+kw)*128 + co_mod]
    wt = np.ascontiguousarray(
        W.reshape(CHUNKS, 128, C_IN, KH, KW).transpose(2, 0, 3, 4, 1)
        .reshape(C_IN, CHUNKS * W_PER_CHUNK).astype(bf16))
    bias = np.ascontiguousarray(b.reshape(CHUNKS, 128).T)

    nc = _get_nc()
    in_maps = [
        {"x": xpad[c * B_PER_CORE:(c + 1) * B_PER_CORE], "wt": wt, "bias": bias}
        for c in range(N_CORES)
    ]
    res = run_bass_kernel_spmd(nc, in_maps, core_ids=list(range(N_CORES)),
                               trace=_trace)
    out = np.concatenate([res.results[c]["out"] for c in range(N_CORES)], axis=0)
    if _trace:
        _CACHE["last_results"] = res
    return out



# revision 8
# speedup vs baseline: 1.1200x; 1.1200x over previous
"""Conv2D 3x3 (stride 1, pad 1) Trainium2 Bass kernel — 1D Winograd F(2,3).

Problem: x (16,128,56,56) f32 * W (256,128,3,3) + b (256,) -> (16,256,56,56) f32

Strategy:
  - Data parallel over batch: 8 cores x 2 images; W/b replicated.
  - 1D Winograd F(2,3) along the W (column) axis: host transforms x into 4
    planes xt_p[ci, 58, 28] (bf16) and W into U[kh,p][ci,co] (bf16). Device
    computes m_p[r,j] = sum_kh U(kh,p)^T @ xt_p[r+kh, :] as 3-matmul PSUM
    accumulations (N=392 = 14 rows x 28 cols per group), then combines
      Y_even = m0+m1+m2+b ,  Y_odd = m1-m2-m3+b
    across Scalar (s=act(m1+b), c2=act(m2)), Vector (p=m0+s, Yo=q-m3) and
    GpSimd (q=s-c2, Ye=p+c2) — one touch per output element per engine, all
    under the PE's ~32.5us of matmul work (vs 47.6us for direct conv).
  - PE work per core: 192 matmuls x N=392 = 75k cycles (2/3 of direct conv).
  - Output written bf16 (tolerance allows; halves out-DMA); host interleaves
    even/odd columns and upcasts to f32.
"""

import os
import sys

for _p in ("/opt/trn_rl_repo", os.path.expanduser("~/.axon_site/_ro/trn_rl_repo")):
    if os.path.isdir(_p) and _p not in sys.path:
        sys.path.insert(0, _p)
        break

import numpy as np
import ml_dtypes

B, C_IN, H, W_SP = 16, 128, 56, 56
C_OUT, KH, KW = 256, 3, 3
N_CORES = 8
B_PER_CORE = B // N_CORES          # 2
CHUNKS = C_OUT // 128              # 2
HP = H + 2                         # 58 padded rows
NJ = 28                            # output column pairs
XT_COLS = HP * NJ                  # 1624 per plane
ROWS_PER_G = 14                    # output rows per group
NG = H // ROWS_PER_G               # 4 groups
NT = ROWS_PER_G * NJ               # 392 = matmul moving N
U_PLANE = 128                      # cols per U plane (co)
U_CHUNK = 4 * KH * U_PLANE         # 1536 cols per chunk: p-major, kh, co
N_WARM = 14                        # pstate-ramp dummy matmuls
P_ORDER = (1, 2, 0, 3)             # m-plane fill order (m1,m2 first for drain)

_CACHE = {}


def _build(null=False):
    from concourse import bacc, mybir
    import concourse.tile as tile

    f32 = mybir.dt.float32
    bf16 = mybir.dt.bfloat16
    Ident = mybir.ActivationFunctionType.Identity
    ADD = mybir.AluOpType.add
    SUB = mybir.AluOpType.subtract

    nc = bacc.Bacc(trn_type="TRN2", name="conv_wino")
    xt_h = nc.dram_tensor("xt", [B_PER_CORE, 4, C_IN, XT_COLS], bf16,
                          kind="ExternalInput")
    w_h = nc.dram_tensor("wt", [C_IN, CHUNKS * U_CHUNK], bf16,
                         kind="ExternalInput")
    b_h = nc.dram_tensor("bias", [128, CHUNKS], f32, kind="ExternalInput")
    # out[img][chunk][co][eo][group][392] bf16
    o_h = nc.dram_tensor("out", [B_PER_CORE, CHUNKS, 128, 2, NG, NT], bf16,
                         kind="ExternalOutput")

    with tile.TileContext(nc) as tc:
        with tc.tile_pool(name="const", bufs=1) as cpool, \
             tc.tile_pool(name="dr", bufs=3) as dpool, \
             tc.tile_pool(name="ps", bufs=8, space="PSUM") as ppool:
            b_sb = cpool.tile([128, CHUNKS], f32)

            if null:
                nc.sync.dma_start(out=b_sb[:, :], in_=b_h[:, :])
                zt = cpool.tile([128, NT], bf16)
                nc.vector.memset(zt[:, :], 0)
                nc.sync.dma_start(out=o_h[0, 0, :, 0, 0, :], in_=zt[:, :])
                nc.finalize()
                return nc

            # PE warmup: dummy matmuls runnable immediately (no DMA dep);
            # ramp the PE pstate while the first xt/U DMAs land.
            warm = cpool.tile([128, 128], bf16)
            nc.gpsimd.memset(warm[:, :], 0)
            wps = ppool.tile([128, NT], f32, name="m")
            for _ in range(N_WARM):
                nc.tensor.matmul(wps[:, :128], warm[:, :], warm[:, :],
                                 start=True, stop=True)

            w_sb = cpool.tile([C_IN, CHUNKS * U_CHUNK], bf16)
            xts = [[cpool.tile([C_IN, XT_COLS], bf16, name=f"xt{i}{p}")
                    for p in range(4)] for i in range(B_PER_CORE)]

            # Critical path: first matmul needs U(chunk0, p=1) + xt[0][1].
            # scalar queue: U pieces + bias;  sync queue: xt planes in use
            # order.
            nc.scalar.dma_start(out=w_sb[:, 384:768],
                                in_=w_h[:, 384:768])            # chunk0 p1
            nc.scalar.dma_start(out=w_sb[:, 768:1536],
                                in_=w_h[:, 768:1536])           # chunk0 p2,p3
            nc.scalar.dma_start(out=w_sb[:, :384], in_=w_h[:, :384])  # c0 p0
            nc.scalar.dma_start(out=b_sb[:, :], in_=b_h[:, :])
            nc.scalar.dma_start(out=w_sb[:, 1536:], in_=w_h[:, 1536:])  # c1
            for i in range(B_PER_CORE):
                for p in P_ORDER:
                    nc.sync.dma_start(out=xts[i][p][:, :], in_=xt_h[i, p, :, :])

            for img in range(B_PER_CORE):
                for chunk in range(CHUNKS):
                    for g in range(NG):
                        ps = {}
                        for p in P_ORDER:
                            ps[p] = ppool.tile([128, NT], f32, name="m")
                            for kh in range(KH):
                                wcol = chunk * U_CHUNK + p * (KH * 128) + kh * 128
                                r0 = (ROWS_PER_G * g + kh) * NJ
                                nc.tensor.matmul(
                                    ps[p][:, :],
                                    w_sb[:, wcol:wcol + 128],
                                    xts[img][p][:, r0:r0 + NT],
                                    start=(kh == 0),
                                    stop=(kh == KH - 1),
                                )
                        s = dpool.tile([128, NT], f32, name="s")
                        c2 = dpool.tile([128, NT], f32, name="c2")
                        pt = dpool.tile([128, NT], f32, name="pt")
                        q = dpool.tile([128, NT], f32, name="q")
                        ye = dpool.tile([128, NT], bf16, name="ye")
                        yo = dpool.tile([128, NT], bf16, name="yo")
                        # scalar: PSUM reads of m1 (with bias) and m2
                        nc.scalar.activation(s[:, :], ps[1][:, :], Ident,
                                             bias=b_sb[:, chunk:chunk + 1])
                        nc.scalar.activation(c2[:, :], ps[2][:, :], Ident)
                        # vector: p = m0 + s ; Ye = p + c2 ; Yo = q - m3
                        nc.vector.tensor_tensor(pt[:, :], ps[0][:, :], s[:, :], ADD)
                        # gpsimd (SBUF only): q = s - c2
                        nc.gpsimd.tensor_tensor(q[:, :], s[:, :], c2[:, :], SUB)
                        nc.vector.tensor_tensor(ye[:, :], pt[:, :], c2[:, :], ADD)
                        nc.vector.tensor_tensor(yo[:, :], q[:, :], ps[3][:, :], SUB)
                        nc.sync.dma_start(out=o_h[img, chunk, :, 0, g, :],
                                          in_=ye[:, :])
                        nc.sync.dma_start(out=o_h[img, chunk, :, 1, g, :],
                                          in_=yo[:, :])
    nc.finalize()
    return nc


def _get_nc(null=False):
    key = ("nc", null)
    if key not in _CACHE:
        _CACHE[key] = _build(null=null)
    return _CACHE[key]


def kernel(x, W, b, _trace=False):
    from concourse.bass_utils import run_bass_kernel_spmd

    x = np.asarray(x, dtype=np.float32)
    W = np.asarray(W, dtype=np.float32)
    b = np.asarray(b, dtype=np.float32)
    bf = ml_dtypes.bfloat16

    # --- host input transform (f32 math, store bf16) ---
    xp = np.zeros((B, C_IN, HP, HP), np.float32)
    xp[:, :, 1:1 + H, 1:1 + W_SP] = x
    c0 = xp[:, :, :, 0:56:2]
    c1 = xp[:, :, :, 1:57:2]
    c2 = xp[:, :, :, 2:58:2]
    c3 = np.zeros_like(c0)
    c3[:, :, :, :27] = xp[:, :, :, 3:57:2]
    # planes p=0..3: [B, CI, 4, 58, 28]
    xt = np.stack([c0 - c2, c1 + c2, c2 - c1, c1 - c3], axis=2).astype(bf)
    xt = np.ascontiguousarray(xt.reshape(B, C_IN, 4, XT_COLS).transpose(0, 2, 1, 3))
    xt = xt.reshape(B, 4, C_IN, XT_COLS)

    # --- host weight transform: U[p,kh][ci,co], layout [ci, chunk,p,kh,co] ---
    G = np.array([[1, 0, 0], [0.5, 0.5, 0.5], [0.5, -0.5, 0.5], [0, 0, 1]],
                 np.float32)
    U = np.einsum("pk,oihk->ihpo", G, W)        # [ci, kh, p, co]
    wt = (U.transpose(0, 2, 1, 3)               # [ci, p, kh, co]
          .reshape(C_IN, 4, KH, CHUNKS, 128)
          .transpose(0, 3, 1, 2, 4)             # [ci, chunk, p, kh, co]
          .reshape(C_IN, CHUNKS * U_CHUNK).astype(bf))
    wt = np.ascontiguousarray(wt)
    bias = np.ascontiguousarray(b.reshape(CHUNKS, 128).T)

    nc = _get_nc()
    in_maps = [
        {"xt": xt[c * B_PER_CORE:(c + 1) * B_PER_CORE], "wt": wt, "bias": bias}
        for c in range(N_CORES)
    ]
    res = run_bass_kernel_spmd(nc, in_maps, core_ids=list(range(N_CORES)),
                               trace=_trace)
    # gather: res out [2, 2, 128, 2, 4, 392] bf16 per core
    full = np.empty((B, C_OUT, H, W_SP), np.float32)
    for c in range(N_CORES):
        o = np.asarray(res.results[c]["out"]).astype(np.float32)
        o = o.reshape(B_PER_CORE, CHUNKS, 128, 2, NG, ROWS_PER_G, NJ)
        # -> [img, chunk, co, group, rows, j, eo]
        o = o.transpose(0, 1, 2, 4, 5, 6, 3)
        full[c * B_PER_CORE:(c + 1) * B_PER_CORE] = o.reshape(
            B_PER_CORE, C_OUT, H, W_SP)
    if _trace:
        _CACHE["last_results"] = res
    return full
